# revision 1
# baseline (speedup 1.0000x reference)
"""Trainium2 Bass kernel for nn_DeconvSlimCapsule3D (ConvTranspose3d capsule
layer with sabour dynamic routing), SPMD across 8 NeuronCores.

Sharding: core c = b*4 + s  (b = batch in {0,1}, s = D-slab in {0..3}).
Each core computes output D-planes [8s, 8s+8) of the 32^3 volume for batch b
from a 6-plane halo'd input slab. Zero inter-core communication.

v2 design (vs v1): fp16 everywhere, phases batched over parity blocks of 4
(NPAR), compact routing layouts, route replication via SB->SB DMA, Newton
rsqrt on DVE (scalar engine only ever runs exp -> ~2 act-table loads/block),
iteration-0 preact via W^T(sum_j x_j).

Layouts per block (BLK = NPAR*1024 positions, chunks of 512, halves of BLK/2):
  votes   [128 caps=(od*16+oa), 8j * BLK]   fp16
  preact  [128 caps, BLK]                   fp16
  compact [128 = 64h+8j+od, HALF]           logits/c1/el/route/R2
  chunked [NCH*8 = 8c+od (or 8c+j), 512]    n1S/Z/rsq1/rZ/G
"""
import numpy as np
import ml_dtypes

B, IN_DIM, OUT_DIM, IN_ATOMS, OUT_ATOMS = 2, 8, 8, 16, 16
K, STRIDE, PAD = 4, 2, 1
CH = IN_ATOMS
D = 16
DO = 32
SLAB = 6 * 18 * 18  # 1944
F16 = np.float16

NPAR = 4               # parities per block
NBLK = 8 // NPAR
BLK = NPAR * 1024
NCH = BLK // 512       # chunks per block
HALF = BLK // 2
NCHH = NCH // 2        # chunks per half
NEWTON_STEPS = 1

_CACHE = {}


# ---------------- host-side prep ----------------

def _k_tap(r, d):
    return 3 - 2 * d if r == 0 else 2 - 2 * d


def _build_wcol(w):
    wcol = np.zeros((8, 128, 128), np.float32)
    for p in range(8):
        rd, rh, rw = p >> 2 & 1, p >> 1 & 1, p & 1
        for t in range(8):
            dd, dh, dw = t >> 2 & 1, t >> 1 & 1, t & 1
            kk = (_k_tap(rd, dd), _k_tap(rh, dh), _k_tap(rw, dw))
            wcol[p, t * 16:t * 16 + 16, :] = w[:, :, kk[0], kk[1], kk[2]]
    return wcol.transpose(1, 0, 2).reshape(128, 8 * 128)


def _make_xrep(x, b, s):
    slab = np.zeros((IN_DIM, CH, 6, 18, 18), np.float32)
    for j0 in range(6):
        i = 4 * s - 1 + j0
        if 0 <= i < D:
            slab[:, :, j0, 1:17, 1:17] = x[b, :, :, i]
    flat = slab.reshape(IN_DIM, CH, SLAB)
    xrep = np.zeros((128, IN_DIM * SLAB), np.float32)
    for t in range(8):
        dd, dh, dw = t >> 2 & 1, t >> 1 & 1, t & 1
        off = dd * 324 + dh * 18 + dw
        n = SLAB - off
        for j in range(IN_DIM):
            xrep[t * 16:t * 16 + 16, j * SLAB:j * SLAB + n] = flat[j, :, off:]
    return xrep


def _host_constants(w, deconv_b, routing_bias):
    oall = np.zeros((128, 16 * 128), np.float32)     # slice s = h*8+j
    for h in range(2):
        for j in range(8):
            s = h * 8 + j
            for od in range(8):
                oall[od * 16:(od + 1) * 16, s * 128 + 64 * h + 8 * j + od] = 1.0
    # chunked row index: r(c) = 8*(c % NCHH) + {od|j}, half h = c // NCHH in
    # partition range [32h, 32h+32) of the [64, 512] accumulators.
    o1ch = np.zeros((128, NCH * 32), np.float32)     # slice c: caps -> r(c)
    oz = np.zeros((128, NCH * 32), np.float32)       # slice c: (h,j,od) -> r(c)
    e2ch = np.zeros((64, NCH * 128), np.float32)     # slice c: 32h+r(c) -> (h,j,od)
    rze = np.zeros((64, NCH * 128), np.float32)      # slice c: 32h+r(c) -> (h,j,od)
    ebigch = np.zeros((64, NCH * 128), np.float32)   # slice c: 32h+r(c) -> caps
    for c in range(NCH):
        h, c4 = c // NCHH, c % NCHH
        for od in range(8):
            o1ch[od * 16:(od + 1) * 16, c * 32 + 8 * c4 + od] = 1.0
            e2ch[32 * h + 8 * c4 + od, c * 128 + 64 * h + 8 * np.arange(8) + od] = 1.0
            ebigch[32 * h + 8 * c4 + od, c * 128 + od * 16:c * 128 + (od + 1) * 16] = 1.0
        for j in range(8):
            oz[64 * h + 8 * j:64 * h + 8 * j + 8, c * 32 + 8 * c4 + j] = 1.0
            rze[32 * h + 8 * c4 + j, c * 128 + 64 * h + 8 * j + np.arange(8)] = 1.0
    i128 = np.eye(128, dtype=np.float32)
    rb = np.broadcast_to(routing_bias.reshape(-1), (128,)).astype(np.float32)
    bias3 = np.stack([deconv_b.astype(np.float32),
                      deconv_b.astype(np.float32) + rb, rb], axis=1)
    return {
        "wcol": _build_wcol(w).astype(F16),
        "oall": oall.astype(F16), "o1ch": o1ch.astype(F16),
        "oz": oz.astype(F16), "e2ch": e2ch.astype(F16),
        "rze": rze.astype(F16), "ebigch": ebigch.astype(F16),
        "i128": i128.astype(F16), "bias3": bias3,
    }


# ---------------- bass kernel ----------------

def _build_nc():
    import concourse.bass as bass
    import concourse.tile as tile
    from concourse import bacc, mybir
    from contextlib import ExitStack

    f32 = mybir.dt.float32
    fp16 = mybir.dt.float16
    i32 = mybir.dt.int32
    AF = mybir.ActivationFunctionType
    ALU = mybir.AluOpType

    nc = bacc.Bacc("TRN2", target_bir_lowering=False, debug=False)

    xrep_d = nc.dram_tensor("xrep", [128, IN_DIM * SLAB], fp16, kind="ExternalInput").ap()
    xsum_d = nc.dram_tensor("xsum", [128, SLAB], fp16, kind="ExternalInput").ap()
    wcol_d = nc.dram_tensor("wcol", [128, 8 * 128], fp16, kind="ExternalInput").ap()
    oall_d = nc.dram_tensor("oall", [128, 16 * 128], fp16, kind="ExternalInput").ap()
    o1ch_d = nc.dram_tensor("o1ch", [128, NCH * 32], fp16, kind="ExternalInput").ap()
    oz_d = nc.dram_tensor("oz", [128, NCH * 32], fp16, kind="ExternalInput").ap()
    e2ch_d = nc.dram_tensor("e2ch", [64, NCH * 128], fp16, kind="ExternalInput").ap()
    rze_d = nc.dram_tensor("rze", [64, NCH * 128], fp16, kind="ExternalInput").ap()
    ebigch_d = nc.dram_tensor("ebigch", [64, NCH * 128], fp16, kind="ExternalInput").ap()
    i128_d = nc.dram_tensor("i128", [128, 128], fp16, kind="ExternalInput").ap()
    bias3_d = nc.dram_tensor("bias3", [128, 3], f32, kind="ExternalInput").ap()
    out_d = nc.dram_tensor("out", [128, 8 * 1024], f32, kind="ExternalOutput").ap()

    def pslice(t, p0, pn, c0, dims):
        # AP over tile t: partitions [p0, p0+pn), free offset c0 elements, dims list
        a = t[:, :]
        return bass.AP(tensor=a.tensor, offset=a.offset + p0 * a.ap[0][0] + c0,
                       ap=[[a.ap[0][0], pn]] + dims)

    with tile.TileContext(nc) as tc, ExitStack() as ctx:
        consts = ctx.enter_context(tc.tile_pool(name="consts", bufs=1))
        xpool = ctx.enter_context(tc.tile_pool(name="xrep", bufs=1))
        vpool = ctx.enter_context(tc.tile_pool(name="votes", bufs=1))
        papool = ctx.enter_context(tc.tile_pool(name="preact", bufs=2))
        cpool = ctx.enter_context(tc.tile_pool(name="compact", bufs=1))   # R2/logits/c1/el/route
        tpool = ctx.enter_context(tc.tile_pool(name="trans", bufs=2))     # pr/sq chunks
        rpool = ctx.enter_context(tc.tile_pool(name="rep", bufs=2))       # rep8 (chunk pair)
        spool = ctx.enter_context(tc.tile_pool(name="small", bufs=1))     # [64,512] temps
        opool = ctx.enter_context(tc.tile_pool(name="out", bufs=1))
        psA = ctx.enter_context(tc.tile_pool(name="psA", bufs=2, space="PSUM"))
        psB = ctx.enter_context(tc.tile_pool(name="psB", bufs=2, space="PSUM"))
        psC = ctx.enter_context(tc.tile_pool(name="psC", bufs=2, space="PSUM"))
        psD = ctx.enter_context(tc.tile_pool(name="psD", bufs=2, space="PSUM"))

        xrep_sb = xpool.tile([128, IN_DIM * SLAB], fp16)
        nc.sync.dma_start(xrep_sb, xrep_d)
        xsum_sb = xpool.tile([128, SLAB], fp16)
        nc.sync.dma_start(xsum_sb, xsum_d)
        wcol_sb = consts.tile([128, 8 * 128], fp16)
        nc.sync.dma_start(wcol_sb, wcol_d)
        oall_sb = consts.tile([128, 16 * 128], fp16)
        nc.sync.dma_start(oall_sb, oall_d)
        o1ch_sb = consts.tile([128, NCH * 32], fp16)
        nc.sync.dma_start(o1ch_sb, o1ch_d)
        oz_sb = consts.tile([128, NCH * 32], fp16)
        nc.sync.dma_start(oz_sb, oz_d)
        e2ch_sb = consts.tile([64, NCH * 128], fp16)
        nc.sync.dma_start(e2ch_sb, e2ch_d)
        rze_sb = consts.tile([64, NCH * 128], fp16)
        nc.sync.dma_start(rze_sb, rze_d)
        ebigch_sb = consts.tile([64, NCH * 128], fp16)
        nc.sync.dma_start(ebigch_sb, ebigch_d)
        i128_sb = consts.tile([128, 128], fp16)
        nc.sync.dma_start(i128_sb, i128_d)
        bias_sb = consts.tile([128, 3], f32)
        nc.sync.dma_start(bias_sb, bias3_d)

        def window(src, j, p, h2):
            rd, rh, rw = p >> 2 & 1, p >> 1 & 1, p & 1
            base = (j * SLAB if j is not None else 0) + rd * 324 + rh * 18 + rw + h2 * 648
            a = src[:, :]
            return bass.AP(tensor=a.tensor, offset=a.offset + base,
                           ap=[list(a.ap[0]), [324, 2], [18, 16], [1, 16]])

        def newton_rsqrt(x_ap, out_ap, t1_t, yy_t, nxt_t, p0, pn):
            # out = rsqrt(x); x_ap [pn, 512] f32 PSUM slice at partition p0.
            # t1/yy/nxt are [64,512]-class tiles sliced at the same base.
            def sl(t, dt=None):
                a = pslice(t, p0, pn, 0, [[1, 512]])
                return a.bitcast(dt) if dt else a
            xi = x_ap.bitcast(i32)
            t1 = sl(t1_t, i32)
            nc.vector.tensor_scalar(t1, xi, 1, None, op0=ALU.arith_shift_right)
            nc.vector.tensor_scalar(t1, t1, -1, 0x5F3759DF, op0=ALU.mult, op1=ALU.add)
            cur = sl(t1_t).bitcast(f32)
            for s in range(NEWTON_STEPS):
                yy = sl(yy_t)
                nc.vector.tensor_mul(yy, cur, cur)
                nc.vector.tensor_mul(yy, x_ap, yy)
                nc.vector.tensor_scalar(yy, yy, -0.5, 1.5, op0=ALU.mult, op1=ALU.add)
                if s == NEWTON_STEPS - 1:
                    nc.vector.tensor_mul(out_ap, cur, yy)
                else:
                    nxt = sl(nxt_t)
                    nc.vector.tensor_mul(nxt, cur, yy)
                    cur = nxt

        NC8 = NCH * 8

        for b in range(NBLK):
            # ---------------- front: deconv + votesum + n2/R2 ----------------
            votes = vpool.tile([128, 8 * BLK], fp16, tag="votes")
            preact = papool.tile([128, BLK], fp16, tag="pa")
            for p4 in range(NPAR):
                p = NPAR * b + p4
                for j in range(8):
                    for h2 in (0, 1):
                        ps = psA.tile([128, 512], f32, tag="big")
                        nc.tensor.matmul(ps, wcol_sb[:, p * 128:(p + 1) * 128],
                                         window(xrep_sb, j, p, h2), start=True, stop=True)
                        vdst = votes[:, j * BLK + p4 * 1024 + h2 * 512:
                                     j * BLK + p4 * 1024 + h2 * 512 + 512]
                        if j % 2 == 0:
                            nc.scalar.activation(vdst, ps, AF.Identity,
                                                 bias=bias_sb[:, 0:1])
                        else:
                            nc.vector.tensor_scalar(vdst, ps, bias_sb[:, 0:1], None,
                                                    op0=ALU.add)
                for h2 in (0, 1):
                    ps = psA.tile([128, 512], f32, tag="big")
                    nc.tensor.matmul(ps, wcol_sb[:, p * 128:(p + 1) * 128],
                                     window(xsum_sb, None, p, h2), start=True, stop=True)
                    nc.scalar.activation(
                        preact[:, p4 * 1024 + h2 * 512:p4 * 1024 + h2 * 512 + 512],
                        ps, AF.Identity, scale=0.125, bias=bias_sb[:, 1:2])

            # n2 + R2  (R2 = rsqrt(n2) in compact [128, HALF], fp16)
            # ln lands in the R2 tile, then exp(-0.5 ln) in place.
            R2 = cpool.tile([128, HALF], fp16, tag="R2")
            for c in range(NCH):
                h, q = c // NCHH, (c % NCHH) * 512
                sq = tpool.tile([128, 8 * 512], fp16, tag="big8")
                va = pslice(votes, 0, 128, c * 512, [[BLK, 8], [1, 512]])
                nc.vector.tensor_mul(
                    sq[:, :].rearrange("p (j n) -> p j n", j=8), va, va)
                psn2 = psB.tile([128, 512], f32, tag="exp")
                for j in range(8):
                    s = h * 8 + j
                    nc.tensor.matmul(psn2, oall_sb[:, s * 128:(s + 1) * 128],
                                     sq[:, j * 512:(j + 1) * 512],
                                     start=(j == 0), stop=(j == 7))
                nc.scalar.activation(pslice(R2, 64 * h, 64, q, [[1, 512]]),
                                     pslice(psn2, 64 * h, 64, 0, [[1, 512]]), AF.Ln)
            nc.scalar.activation(R2, R2, AF.Exp, scale=-0.5)

            # ---------------- routing iterations ----------------
            logits = cpool.tile([128, HALF], fp16, tag="logits")
            el = None
            for it in (1, 2):
                # stage A: sqp/n1S + pr/dot/c1 per chunk
                c1 = cpool.tile([128, HALF], fp16, tag="c1")
                psn1 = psD.tile([64, 512], f32, tag="acc")
                rsq1 = spool.tile([64, 512], fp16, tag="rsq1")
                nw_t1 = spool.tile([64, 512], i32, tag="rz")
                nw_yy = spool.tile([64, 512], f32, tag="nw2")
                nw_nx = spool.tile([64, 512], f32, tag="nw5") if NEWTON_STEPS > 1 else None
                for c in range(NCH):
                    h, q = c // NCHH, (c % NCHH) * 512
                    sqp = tpool.tile([128, 512], fp16, tag="sqp", bufs=2)
                    nc.vector.tensor_mul(sqp, preact[:, c * 512:(c + 1) * 512],
                                         preact[:, c * 512:(c + 1) * 512])
                    nc.tensor.matmul(pslice(psn1, 32 * h, 32, 0, [[1, 512]]),
                                     o1ch_sb[:, c * 32:(c + 1) * 32], sqp,
                                     start=(c % NCHH == 0), stop=(c % NCHH == NCHH - 1))
                    pr = tpool.tile([128, 8 * 512], fp16, tag="big8")
                    va = pslice(votes, 0, 128, c * 512, [[BLK, 8], [1, 512]])
                    pb = pslice(preact, 0, 128, c * 512, [[0, 8], [1, 512]])
                    nc.vector.tensor_mul(
                        pr[:, :].rearrange("p (j n) -> p j n", j=8), va, pb)
                    psdot = psA.tile([128, 512], f32, tag="big")
                    for j in range(8):
                        s = h * 8 + j
                        nc.tensor.matmul(psdot, oall_sb[:, s * 128:(s + 1) * 128],
                                         pr[:, j * 512:(j + 1) * 512],
                                         start=(j == 0), stop=(j == 7))
                    nc.vector.tensor_mul(pslice(c1, 64 * h, 64, q, [[1, 512]]),
                                         pslice(psdot, 64 * h, 64, 0, [[1, 512]]),
                                         pslice(R2, 64 * h, 64, q, [[1, 512]]))
                    if c % NCHH == NCHH - 1:
                        newton_rsqrt(pslice(psn1, 32 * h, 32, 0, [[1, 512]]),
                                     pslice(rsq1, 32 * h, 32, 0, [[1, 512]]),
                                     nw_t1, nw_yy, nw_nx, 32 * h, 32)
                # stage B: rsq1e, logits, el, Z
                el = cpool.tile([128, HALF], fp16, tag="el")
                psz = psD.tile([64, 512], f32, tag="acc")
                rz = spool.tile([64, 512], fp16, tag="rzz")
                rzf = spool.tile([64, 512], f32, tag="nw6")
                for c in range(NCH):
                    h, q = c // NCHH, (c % NCHH) * 512
                    psr1 = psB.tile([128, 512], f32, tag="exp")
                    nc.tensor.matmul(psr1,
                                     pslice(e2ch_sb, 32 * h, 32, c * 128, [[1, 128]]),
                                     pslice(rsq1, 32 * h, 32, 0, [[1, 512]]),
                                     start=True, stop=True)
                    if it == 1:
                        nc.vector.tensor_mul(pslice(logits, 64 * h, 64, q, [[1, 512]]),
                                             pslice(c1, 64 * h, 64, q, [[1, 512]]),
                                             pslice(psr1, 64 * h, 64, 0, [[1, 512]]))
                    else:
                        nc.vector.tensor_mul(pslice(c1, 64 * h, 64, q, [[1, 512]]),
                                             pslice(c1, 64 * h, 64, q, [[1, 512]]),
                                             pslice(psr1, 64 * h, 64, 0, [[1, 512]]))
                        nc.vector.tensor_add(pslice(logits, 64 * h, 64, q, [[1, 512]]),
                                             pslice(logits, 64 * h, 64, q, [[1, 512]]),
                                             pslice(c1, 64 * h, 64, q, [[1, 512]]))
                    nc.scalar.activation(pslice(el, 64 * h, 64, q, [[1, 512]]),
                                         pslice(logits, 64 * h, 64, q, [[1, 512]]), AF.Exp)
                    nc.tensor.matmul(pslice(psz, 32 * h, 32, 0, [[1, 512]]),
                                     pslice(oz_sb, 64 * h, 64, c * 32, [[1, 32]]),
                                     pslice(el, 64 * h, 64, q, [[1, 512]]),
                                     start=(c % NCHH == 0), stop=(c % NCHH == NCHH - 1))
                    if c % NCHH == NCHH - 1:
                        nc.vector.reciprocal(pslice(rzf, 32 * h, 32, 0, [[1, 512]]),
                                             pslice(psz, 32 * h, 32, 0, [[1, 512]]))
                        nc.vector.tensor_copy(pslice(rz, 32 * h, 32, 0, [[1, 512]]),
                                              pslice(rzf, 32 * h, 32, 0, [[1, 512]]))
                # stage C: route, rep (DMA), prods, jsum -> preact'
                route = cpool.tile([128, HALF], fp16, tag="route")
                for c in range(NCH):
                    h, q = c // NCHH, (c % NCHH) * 512
                    psrz = psB.tile([128, 512], f32, tag="exp")
                    nc.tensor.matmul(psrz,
                                     pslice(rze_sb, 32 * h, 32, c * 128, [[1, 128]]),
                                     pslice(rz, 32 * h, 32, 0, [[1, 512]]),
                                     start=True, stop=True)
                    nc.vector.tensor_mul(pslice(route, 64 * h, 64, q, [[1, 512]]),
                                         pslice(el, 64 * h, 64, q, [[1, 512]]),
                                         pslice(psrz, 64 * h, 64, 0, [[1, 512]]))
                preact_new = papool.tile([128, BLK], fp16, tag="pa")
                for c2 in range(NCH // 2):
                    h, q = c2 // (NCHH // 2), (c2 % (NCHH // 2)) * 1024
                    rep8 = rpool.tile([128, 8 * 1024], fp16, tag="rep")
                    ra = route[:, :]
                    for j in range(8):
                        src = bass.AP(tensor=ra.tensor,
                                      offset=ra.offset + (64 * h + 8 * j) * ra.ap[0][0] + q,
                                      ap=[[ra.ap[0][0], 8], [0, 16], [1, 1024]])
                        nc.gpsimd.dma_start(rep8[:, j * 1024:(j + 1) * 1024], src)
                    for g in (0, 1):
                        c = c2 * 2 + g
                        va = pslice(votes, 0, 128, c * 512, [[BLK, 8], [1, 512]])
                        rep_v = pslice(rep8, 0, 128, g * 512, [[1024, 8], [1, 512]])
                        # in-place: rep8 half becomes prods
                        nc.vector.tensor_mul(rep_v, va, rep_v)
                        pssum = psC.tile([128, 512], f32, tag="sum")
                        for j in range(8):
                            nc.tensor.matmul(pssum, i128_sb,
                                             rep8[:, j * 1024 + g * 512:
                                                  j * 1024 + g * 512 + 512],
                                             start=(j == 0), stop=(j == 7))
                        nc.scalar.activation(preact_new[:, c * 512:(c + 1) * 512], pssum,
                                             AF.Identity, bias=bias_sb[:, 2:3])
                preact = preact_new

            # ---------------- squash + output ----------------
            psnn = psD.tile([64, 512], f32, tag="acc")
            rsqn = spool.tile([64, 512], fp16, tag="rsq1")
            nw_t1 = spool.tile([64, 512], i32, tag="rz")
            nw_yy = spool.tile([64, 512], f32, tag="nw2")
            nw_nx = spool.tile([64, 512], f32, tag="nw5") if NEWTON_STEPS > 1 else None
            G = spool.tile([64, 512], fp16, tag="rzz")
            dd = spool.tile([64, 512], f32, tag="nw6")
            for c in range(NCH):
                h = c // NCHH
                sqs = tpool.tile([128, 512], fp16, tag="sqp", bufs=2)
                nc.vector.tensor_mul(sqs, preact[:, c * 512:(c + 1) * 512],
                                     preact[:, c * 512:(c + 1) * 512])
                nc.tensor.matmul(pslice(psnn, 32 * h, 32, 0, [[1, 512]]),
                                 o1ch_sb[:, c * 32:(c + 1) * 32], sqs,
                                 start=(c % NCHH == 0), stop=(c % NCHH == NCHH - 1))
                if c % NCHH == NCHH - 1:
                    nn = pslice(psnn, 32 * h, 32, 0, [[1, 512]])
                    newton_rsqrt(nn, pslice(rsqn, 32 * h, 32, 0, [[1, 512]]),
                                 nw_t1, nw_yy, nw_nx, 32 * h, 32)
                    nrm = pslice(nw_yy, 32 * h, 32, 0, [[1, 512]])
                    nc.vector.tensor_mul(nrm, nn, pslice(rsqn, 32 * h, 32, 0, [[1, 512]]))
                    ddh = pslice(dd, 32 * h, 32, 0, [[1, 512]])
                    nc.vector.tensor_scalar(ddh, nn, 1.0, None, op0=ALU.add)
                    nc.vector.reciprocal(ddh, ddh)
                    nc.vector.tensor_mul(pslice(G, 32 * h, 32, 0, [[1, 512]]), nrm, ddh)
            for c in range(NCH):
                h = c // NCHH
                psg = psB.tile([128, 512], f32, tag="exp")
                nc.tensor.matmul(psg,
                                 pslice(ebigch_sb, 32 * h, 32, c * 128, [[1, 128]]),
                                 pslice(G, 32 * h, 32, 0, [[1, 512]]),
                                 start=True, stop=True)
                outt = opool.tile([128, 512], f32, tag="out")
                nc.vector.tensor_mul(outt, preact[:, c * 512:(c + 1) * 512], psg)
                nc.sync.dma_start(out_d[:, b * BLK + c * 512:b * BLK + c * 512 + 512],
                                  outt)

    nc.compile()
    return nc


# ---------------- public entry point ----------------

def kernel(x, w, deconv_b, routing_bias):
    from concourse.bass_utils import run_bass_kernel_spmd

    x = np.asarray(x, np.float32)
    w = np.asarray(w, np.float32)
    deconv_b = np.asarray(deconv_b, np.float32)
    routing_bias = np.asarray(routing_bias, np.float32)

    if "nc" not in _CACHE:
        _CACHE["nc"] = _build_nc()
    nc = _CACHE["nc"]

    consts = _host_constants(w, deconv_b, routing_bias)
    in_maps = []
    for c in range(8):
        b, s = c // 4, c % 4
        m = dict(consts)
        xr = _make_xrep(x, b, s)
        m["xrep"] = xr.astype(F16)
        m["xsum"] = xr.reshape(128, IN_DIM, SLAB).sum(axis=1).astype(F16)
        in_maps.append(m)

    res = run_bass_kernel_spmd(nc, in_maps, list(range(8)),
                               trace=bool(_CACHE.get("trace")),
                               tmpdir=_CACHE.get("trace_tmpdir"))
    _CACHE["last_res"] = res

    out = np.zeros((B, OUT_DIM, OUT_ATOMS, DO, DO, DO), np.float32)
    for c in range(8):
        b, s = c // 4, c % 4
        blk = np.asarray(res.results[c]["out"], np.float32)
        blk = blk.reshape(OUT_DIM, OUT_ATOMS, 2, 2, 2, 4, 16, 16)
        t = blk.transpose(0, 1, 5, 2, 6, 3, 7, 4)  # od,oa,a',rd,bh,rh,bw,rw
        out[b, :, :, 8 * s:8 * s + 8, :, :] = t.reshape(OUT_DIM, OUT_ATOMS, 8, 32, 32)
    return out

